# revision 46
# baseline (speedup 1.0000x reference)
"""Trainium2 Bass kernel for nn_NewtonLoss (segment_reduce).

Computes, for K refinement states over N atoms grouped into M molecules:
    sq[k,i]   = ||states_x[k,i,:] - x_target[i,:]||^2
    S[m,k]    = segment_sum(sq[k], molecule_id)
    per_state = sum_m valid_m * S[m,k]/c_m / V
    loss      = sum_k w_k * per_state_k        (w = normalized gamma powers)

Default strategy (variant "uw8"/"uw16", 8-core SPMD, memory-bound):
  - The segment reduce collapses to a weighted dot product: per-atom
    weight w_i = 1/count(molecule(i)), so loss = sum_{k,i} w_k*w_i/V *
    ||st[k,i]-tgt[i]||^2 — order-independent, no scan, no contiguity.
  - Host stages each core's 250k-atom shard atom-major [P, R, K, 3] in
    fp8e4m3 (uw8) or fp16 (uw16), target and sqrt(w) (x3) in fp16, so
    every chunk DMA is one long contiguous line per partition.
  - On device per chunk: diff = st - tgt (tensor_tensor, fp16 2x DVE /
    GPSIMD split by k), dw = diff*sqrt_w (same), then per (chunk, k)
    ACT Square(accum_out) or DVE STT accumulate: acc = sum(dw^2).
  - Every rep's accumulators are DMA'd to a distinct HBM block (keeps
    the repeat loop live for honest marginal timing).
  - Host sums the 8 tiny per-core accumulators into the final scalar.

Host-side work on the big arrays is staging only (shard/layout/dtype);
all arithmetic on states/target runs on the NeuronCores. The old
scan-based variant is kept as build_program(variant="scan").
"""

import os
import sys

import numpy as np

for _p in ("/opt/trn_rl_repo",):
    if os.path.isdir(_p) and _p not in sys.path:
        sys.path.insert(0, _p)

import concourse.bacc as bacc  # noqa: E402
import concourse.bass as bass  # noqa: E402
import concourse.tile as tile  # noqa: E402
from concourse import mybir  # noqa: E402

GAMMA = 0.7
NCORES = 8
P = 128  # partitions

# Full-problem geometry (N = 2_000_000 atoms):
#   per-core window = NTILES * P * R atoms; overlapping windows cover N.
K_FULL = 8
R_FULL = 128          # atoms per partition-row per tile
NTILES_FULL = 16
CHUNK_TILES_FULL = 4  # tiles per scan chunk

# v2 ("uw16") geometry: per-core shard = P * R2 atoms, fp16 host staging.
R2_FULL = 1960        # atoms per partition; SHARD2 = 250_880 >= 250_000
NCHUNKS2_FULL = 4

DEFAULT_VARIANT = "uw16"


def build_program(K=K_FULL, ntiles=NTILES_FULL, R=R_FULL,
                  chunk_tiles=CHUNK_TILES_FULL, reps=1, variant="scan",
                  add1_engine="gpsimd", add2_engine="vector",
                  red_mode="act", stbufs=3, chbufs=2,
                  scan_engine="vector", mul_engine="vector",
                  stop_after="full", cast=True, dropmask=False,
                  num_devices=1):
    """Build the single-core Bass program (run SPMD on all cores).

    variant "scan": masked segmented scan + piece-end weights (W sparse).
    variant "uw":   per-atom 1/count weights, no scan (W dense).
    red_mode "ttr": fused multiply-reduce on DVE.
    red_mode "act": multiply on DVE, accumulate via ACT Copy(accum_out).
    """
    TILE = P * R
    SHARD = ntiles * TILE
    RD = R * 3
    nchunks = ntiles // chunk_tiles
    CH = chunk_tiles * R  # scan length per chunk per partition
    f32 = mybir.dt.float32
    f16 = mybir.dt.float16 if cast else f32
    add, mult = mybir.AluOpType.add, mybir.AluOpType.mult

    nc = bacc.Bacc("TRN2", target_bir_lowering=False, debug=False,
                   num_devices=num_devices)
    states = nc.dram_tensor("states", [K, SHARD, 3], f32, kind="ExternalInput").ap()
    target = nc.dram_tensor("target", [SHARD, 3], f32, kind="ExternalInput").ap()
    maskd = (None if dropmask else
             nc.dram_tensor("mask", [SHARD], f32, kind="ExternalInput").ap())
    wvecd = nc.dram_tensor("wvec", [SHARD], f32, kind="ExternalInput").ap()
    accd = nc.dram_tensor("acc", [P, nchunks * K], f32, kind="ExternalOutput").ap()

    # atom i lives at (tile t, partition p, row-pos r): i = t*TILE + p*R + r
    st_v = states.rearrange("k (t p r) d -> t p k (r d)", t=ntiles, p=P)
    tg_v = target.rearrange("(t p r) d -> p t (r d)", t=ntiles, p=P)
    mk_v = (None if dropmask else
            maskd.rearrange("(t p r) -> p t r", t=ntiles, p=P))
    wv_v = wvecd.rearrange("(t p r) -> p t r", t=ntiles, p=P)

    engines = {"vector": nc.vector, "gpsimd": nc.gpsimd}
    add1_e, add2_e = engines[add1_engine], engines[add2_engine]
    mul_e = engines[mul_engine]

    def scan_e(k):
        if scan_engine == "split":
            return nc.vector if k % 2 == 0 else nc.gpsimd
        return engines[scan_engine]

    with tile.TileContext(nc) as tc:
        with (
            tc.tile_pool(name="singles", bufs=1) as singles,
            tc.tile_pool(name="stp", bufs=stbufs) as stp,
            tc.tile_pool(name="dfp", bufs=2) as dfp,
            tc.tile_pool(name="sqp", bufs=2) as sqp,
            tc.tile_pool(name="tmpp", bufs=2) as tmpp,
            tc.tile_pool(name="chp", bufs=chbufs) as chp,
            tc.tile_pool(name="scp", bufs=2) as scp,
            tc.tile_pool(name="ttp", bufs=2) as ttp,
        ):
            tg_all = singles.tile([P, ntiles, RD], f16)
            (nc.gpsimd if cast else nc.sync).dma_start(out=tg_all, in_=tg_v)
            wv_all = singles.tile([P, ntiles, R], f32)
            nc.sync.dma_start(out=wv_all, in_=wv_v)
            if not dropmask:
                # load mask even if unused: a declared-but-stripped input
                # tensor crashes the pjrt exec path
                mk_all = singles.tile([P, ntiles, R], f32)
                nc.sync.dma_start(out=mk_all, in_=mk_v)
            acc = singles.tile([P, nchunks * K], f32)
            order = ["dma", "sub", "sq", "adds", "scan", "full"]
            lvl = order.index(stop_after)
            if lvl < 5:
                nc.vector.memset(acc, 0.0)

            for _rep in range(reps):
                for ch in range(nchunks):
                    sqbuf = chp.tile([P, K, chunk_tiles, R], f32)
                    for j in range(chunk_tiles):
                        t = ch * chunk_tiles + j
                        st = stp.tile([P, K, RD], f16)
                        (nc.gpsimd if cast else nc.sync).dma_start(
                            out=st, in_=st_v[t])
                        if lvl < 1:
                            continue
                        diff = dfp.tile([P, K, RD], f16)
                        tgs = tg_all[:, t, :]
                        tgb = bass.AP(
                            tensor=tgs.tensor, offset=tgs.offset,
                            ap=[list(tgs.ap[0]), [0, K], list(tgs.ap[-1])],
                        )
                        nc.vector.tensor_sub(diff, st, tgb)
                        if lvl < 2:
                            continue
                        sq = sqp.tile([P, K, RD], f32)
                        nc.scalar.square(sq, diff)
                        if lvl < 3:
                            continue
                        sq4 = sq.rearrange("p k (r d) -> p k r d", d=3)
                        tmp = tmpp.tile([P, K, R], f32)
                        add1_e.tensor_add(tmp, sq4[:, :, :, 0], sq4[:, :, :, 1])
                        add2_e.tensor_add(sqbuf[:, :, j, :], tmp, sq4[:, :, :, 2])
                    if lvl < 4:
                        continue
                    wv_ch = wv_all[:, ch * chunk_tiles:(ch + 1) * chunk_tiles, :]
                    wv_ch = wv_ch.rearrange("p t r -> p (t r)")
                    if variant == "scan":
                        mk_ch = mk_all[:, ch * chunk_tiles:(ch + 1) * chunk_tiles, :]
                        mk_ch = mk_ch.rearrange("p t r -> p (t r)")
                    for k in range(K):
                        red_in = sqbuf[:, k, :, :].rearrange("p t r -> p (t r)")
                        if variant == "scan":
                            scano = scp.tile([P, CH], f32)
                            scan_e(k).tensor_tensor_scan(
                                out=scano, data0=mk_ch, data1=red_in,
                                initial=0.0, op0=mult, op1=add)
                            red_in = scano
                        if lvl < 5:
                            continue
                        tto = ttp.tile([P, CH], f32)
                        acc_slot = acc[:, ch * K + k: ch * K + k + 1]
                        if red_mode == "stt":
                            mul_e.scalar_tensor_tensor(
                                out=tto, in0=red_in, scalar=1.0, in1=wv_ch,
                                op0=mult, op1=mult, accum_out=acc_slot)
                        elif red_mode == "act":
                            mul_e.tensor_mul(tto, red_in, wv_ch)
                            nc.scalar.activation(
                                tto, tto, mybir.ActivationFunctionType.Copy,
                                accum_out=acc_slot)
                        else:
                            nc.vector.tensor_tensor_reduce(
                                out=tto, in0=red_in, in1=wv_ch, scale=1.0,
                                scalar=0.0, op0=mult, op1=add,
                                accum_out=acc_slot)
            nc.sync.dma_start(out=accd, in_=acc)
    nc.compile()
    return nc


def build_program_v2(K=K_FULL, R=R2_FULL, nchunks=NCHUNKS2_FULL, reps=1,
                     pw=0, ga=8, stop="full", outq="scalar", accbufs=2,
                     stbufs=2, dfbufs=2, dwbufs=2, st8=False, dmaq="one",
                     whole=False, num_devices=1):
    """v2 single-core program (fp16, no scan): variant "uw16".

    loss contribution = sum_i w_i * ||st[k,i]-tgt[i]||^2 with per-atom
    weights w_i = 1/molecule_count (order-independent weighted dot).
    HBM layout (host-staged fp16): states [P, K, R*3], target [P, R*3],
    wvec3 [P, R*3] carrying sqrt(w) replicated x3.

    Per chunk (engine menu: DVE tensor_tensor runs 2x with fp16;
    scalar_tensor_tensor/reduce are 1x; ACT Square fuses square+accum):
      diff = st - tgt_broadcast      TT sub    (DVE; k<psub on GPSIMD)
      dw   = diff * sqw3_broadcast   TT mult   (k<pw on GPSIMD, rest DVE)
      per k: acc[ch,k] = sum(dw_k^2) ACT Square(accum_out) for k<ga,
                                     DVE STT(dw,dw,mult,accum) for k>=ga
    stop: "dma"|"sub"|"wmult"|"full" — truncate pipeline for stage benches.
    """
    assert R % nchunks == 0
    RC = R // nchunks
    R3 = R * 3
    K3 = K * 3
    f32, f16 = mybir.dt.float32, mybir.dt.float16
    mult, sub = mybir.AluOpType.mult, mybir.AluOpType.subtract
    lvl = ["dma", "sub", "wmult", "full"].index(stop)
    AGRP = 4  # reps per acc-output DMA group

    nc = bacc.Bacc("TRN2", target_bir_lowering=False, debug=False,
                   num_devices=num_devices)
    stdt = mybir.dt.float8e4 if st8 else f16
    # atom-major states layout: per partition, (r, k, d) contiguous ->
    # one long contiguous line per (partition, chunk) = few descriptors
    states = nc.dram_tensor("states", [P, R, K3], stdt, kind="ExternalInput").ap()
    target = nc.dram_tensor("target", [P, R3], f16, kind="ExternalInput").ap()
    wvec3 = nc.dram_tensor("wvec3", [P, R3], f16, kind="ExternalInput").ap()
    # per-rep output blocks: keep every rep's accums live (no DCE of the
    # repeat loop) at negligible DMA cost; combine() reads the last block
    accd = nc.dram_tensor("acc", [P, reps * nchunks * K], f32,
                          kind="ExternalOutput").ap()

    with tile.TileContext(nc) as tc:
        with (
            tc.tile_pool(name="singles", bufs=1) as singles,
            tc.tile_pool(name="stp", bufs=stbufs) as stp,
            tc.tile_pool(name="dfpp", bufs=dfbufs) as dfpp,
            tc.tile_pool(name="dfvp", bufs=dfbufs) as dfvp,
            tc.tile_pool(name="dwpp", bufs=dwbufs) as dwpp,
            tc.tile_pool(name="dwvp", bufs=dwbufs) as dwvp,
            tc.tile_pool(name="ttp", bufs=2) as ttp,
            tc.tile_pool(name="accp", bufs=2) as accp,
        ):
            tg_all = singles.tile([P, R3], f16)
            nc.sync.dma_start(out=tg_all, in_=target)
            wv_all = singles.tile([P, R3], f16)
            nc.sync.dma_start(out=wv_all, in_=wvec3)
            out_eng = {"sync": nc.sync, "scalar": nc.scalar,
                       "gpsimd": nc.gpsimd}[outq]

            def kdbcast(ap2d, r0, n):
                # [P, (r d)] row-major slice -> AP [P, (r:RC), (0,n), (d:3)]
                s = ap2d[:, r0 * 3:(r0 + RC) * 3]
                return bass.AP(tensor=s.tensor, offset=s.offset,
                               ap=[list(s.ap[0]), [3, RC], [0, n], [1, 3]])

            acc = None
            for _rep in range(reps):
                if _rep % AGRP == 0:
                    na = min(AGRP, reps - _rep)
                    acc = accp.tile([P, na, nchunks * K], f32)
                if whole:
                    # one DMA instruction per rep for the whole shard:
                    # probes/avoids per-DMA-instruction fixed costs.
                    # cap per-descriptor payload below the 64 KB SDMA limit
                    stw = stp.tile([P, R, K3], f16)
                    (nc.gpsimd if st8 else nc.sync).dma_start(
                        out=stw, in_=states, max_dma_last_dim=16384)
                for ch in range(nchunks):
                    r0 = ch * RC
                    if whole:
                        st = stw[:, r0:r0 + RC, :]
                    else:
                        st = stp.tile([P, RC, K3], f16)
                        # fp8 HBM -> fp16 SBUF casts go through the gpsimd
                        # SWDGE; dmaq="alt" stripes over both HWDGE queues
                        if st8:
                            steng = nc.gpsimd
                        elif dmaq == "alt":
                            steng = nc.sync if ch % 2 == 0 else nc.scalar
                        else:
                            steng = nc.sync
                        steng.dma_start(out=st, in_=states[:, r0:r0 + RC, :])
                    if lvl < 1:
                        continue

                    st4 = st.rearrange("p r (k d) -> p r k d", d=3)
                    # pool-owned k range [0, pw): sub+wmult on GPSIMD into
                    # its own tiles so ACT accums for those k don't wait on
                    # the DVE chain (and vice versa)
                    pg = pw
                    if pg > 0:
                        dfP = dfpp.tile([P, RC, pg, 3], f16)
                        dwP = dwpp.tile([P, RC, pg, 3], f16)
                        nc.gpsimd.tensor_sub(dfP, st4[:, :, :pg, :],
                                             kdbcast(tg_all, r0, pg))
                        nc.gpsimd.tensor_mul(dwP, dfP,
                                             kdbcast(wv_all, r0, pg))
                    if pg < K:
                        dfV = dfvp.tile([P, RC, K - pg, 3], f16)
                        dwV = dwvp.tile([P, RC, K - pg, 3], f16)
                        nc.vector.tensor_sub(dfV, st4[:, :, pg:, :],
                                             kdbcast(tg_all, r0, K - pg))
                        if lvl < 2:
                            continue
                        nc.vector.tensor_mul(dwV, dfV,
                                             kdbcast(wv_all, r0, K - pg))
                    if lvl < 3:
                        continue

                    for k in range(K):
                        tto = ttp.tile([P, RC, 3], f16)
                        slot = acc[:, _rep % AGRP,
                                   ch * K + k: ch * K + k + 1]
                        dwk = (dwP[:, :, k, :] if k < pg
                               else dwV[:, :, k - pg, :])
                        if k < ga:
                            nc.scalar.activation(
                                tto, dwk,
                                mybir.ActivationFunctionType.Square,
                                accum_out=slot)
                        else:
                            nc.vector.scalar_tensor_tensor(
                                out=tto, in0=dwk, scalar=1.0,
                                in1=dwk, op0=mult, op1=mult,
                                accum_out=slot)
                if _rep % AGRP == AGRP - 1 or _rep == reps - 1:
                    g0 = (_rep // AGRP) * AGRP
                    na = _rep - g0 + 1
                    o0 = g0 * nchunks * K
                    out_eng.dma_start(
                        out=accd[:, o0:o0 + na * nchunks * K],
                        in_=acc[:, :na].rearrange("p a c -> p (a c)"))
    nc.compile()
    return nc


def host_prep_v2(states_x, x_target, molecule_id, num_molecules,
                 ncores=NCORES, K=K_FULL, R=R2_FULL, st8=False):
    """Shard + stage inputs as fp16 (states optionally fp8e4m3) in
    [P, K, R*3] / [P, R*3] layout."""
    import ml_dtypes
    st_dt = ml_dtypes.float8_e4m3 if st8 else np.float16
    SHARD = P * R
    N = molecule_id.shape[0]
    M = int(num_molecules)
    assert N % ncores == 0
    OWN = N // ncores
    assert SHARD >= OWN

    ids = np.asarray(molecule_id).astype(np.int64)
    counts = np.bincount(ids, minlength=M)
    V = int((counts > 0).sum())
    inv_c = np.zeros(M, np.float32)
    nz = counts > 0
    inv_c[nz] = 1.0 / counts[nz]
    w_full = inv_c[ids]  # (N,) fp32

    states_x = np.asarray(states_x)
    x_target = np.asarray(x_target)

    nfull = OWN // R            # full partitions per shard
    rem = OWN - nfull * R       # atoms in the last (padded) partition

    in_maps = []
    for c in range(ncores):
        lo, hi = c * OWN, (c + 1) * OWN
        mid = lo + nfull * R

        # atom-major: [P, R, K, 3] so each (partition, chunk) DMA line is
        # one long contiguous run (minimal descriptor count)
        st = np.zeros((P, R, K, 3), st_dt)
        src = states_x[:, lo:mid, :].astype(st_dt)
        st[:nfull] = src.reshape(K, nfull, R, 3).transpose(1, 2, 0, 3)
        if rem:
            st[nfull, :rem] = states_x[:, mid:hi, :].astype(st_dt).transpose(1, 0, 2)

        tg = np.zeros((P, R, 3), np.float16)
        tg[:nfull] = x_target[lo:mid].astype(np.float16).reshape(nfull, R, 3)
        if rem:
            tg[nfull, :rem, :] = x_target[mid:hi]

        w = np.zeros((P, R), np.float32)
        w[:nfull] = w_full[lo:mid].reshape(nfull, R)
        if rem:
            w[nfull, :rem] = w_full[mid:hi]
        # stage sqrt(w): device computes sum((diff*sqrt_w)^2) = sum(w*diff^2)
        w3 = np.broadcast_to(np.sqrt(w)[:, :, None], (P, R, 3)).astype(np.float16)

        in_maps.append({
            "states": st.reshape(P, R, K * 3),
            "target": np.ascontiguousarray(tg.reshape(P, R * 3)),
            "wvec3": np.ascontiguousarray(w3.reshape(P, R * 3)),
        })
    return in_maps, V


def host_prep(states_x, x_target, molecule_id, num_molecules,
              ncores=NCORES, K=K_FULL, ntiles=NTILES_FULL, R=R_FULL,
              variant=DEFAULT_VARIANT):
    if variant in ("uw16", "uw8"):
        return host_prep_v2(states_x, x_target, molecule_id, num_molecules,
                            ncores=ncores, K=K, st8=(variant == "uw8"))
    return _host_prep_v1(states_x, x_target, molecule_id, num_molecules,
                         ncores=ncores, K=K, ntiles=ntiles, R=R,
                         variant=variant)


def _host_prep_v1(states_x, x_target, molecule_id, num_molecules,
                  ncores=NCORES, K=K_FULL, ntiles=NTILES_FULL, R=R_FULL,
                  variant="scan"):
    """Shard inputs into per-core windows; build mask/weight vectors.

    Returns (in_maps, V) where in_maps[c] are the named inputs for core c.
    """
    TILE = P * R
    SHARD = ntiles * TILE
    N = molecule_id.shape[0]
    M = int(num_molecules)
    assert N % ncores == 0
    OWN = N // ncores
    assert SHARD >= OWN, (SHARD, OWN)

    ids = np.asarray(molecule_id).astype(np.int64)
    counts = np.bincount(ids, minlength=M)
    V = int((counts > 0).sum())
    inv_c = np.zeros(M, np.float64)
    nz = counts > 0
    inv_c[nz] = 1.0 / counts[nz]

    states_x = np.asarray(states_x)
    x_target = np.asarray(x_target)

    r_idx = np.arange(SHARD, dtype=np.int64) % R

    in_maps = []
    for c in range(ncores):
        S_c = 0 if ncores == 1 else (c * (N - SHARD)) // (ncores - 1)
        own_lo, own_hi = c * OWN - S_c, (c + 1) * OWN - S_c
        assert own_lo >= 0 and own_hi <= SHARD

        idw = ids[S_c:S_c + SHARD]
        pos = np.arange(SHARD, dtype=np.int64)
        owned = (pos >= own_lo) & (pos < own_hi)

        if variant == "uw":
            m = np.zeros(SHARD, np.float32)
            w = np.where(owned, inv_c[idw], 0.0)
        else:
            same_prev = np.zeros(SHARD, bool)
            same_prev[1:] = idw[1:] == idw[:-1]
            m = (r_idx > 0) & same_prev & owned
            m[1:] &= owned[:-1]

            nxt_same = np.zeros(SHARD, bool)
            nxt_same[:-1] = idw[:-1] == idw[1:]
            nxt_same[:-1] &= owned[1:]
            nxt_same &= r_idx < (R - 1)
            w = np.where(owned & ~nxt_same, inv_c[idw], 0.0)

        in_maps.append({
            "states": np.ascontiguousarray(states_x[:, S_c:S_c + SHARD, :],
                                           dtype=np.float32),
            "target": np.ascontiguousarray(x_target[S_c:S_c + SHARD, :],
                                           dtype=np.float32),
            "mask": np.asarray(m, np.float32),
            "wvec": np.asarray(w, np.float32),
        })
    return in_maps, V


def combine(results, V, K=K_FULL, nchunks=NCHUNKS2_FULL):
    """Sum per-core accumulators into the final scalar loss."""
    total = np.zeros(K, np.float64)
    for r in results:
        acc = np.asarray(r["acc"]).astype(np.float64)
        if nchunks * K < acc.shape[-1]:  # [P, reps*nchunks*K]
            acc = acc[:, -nchunks * K:]  # last rep's blocks
        total += acc.reshape(P, -1, K).sum(axis=(0, 1))
    per_state = total / V
    w = GAMMA ** ((K - 1) - np.arange(K, dtype=np.float64))
    w = w / w.sum()
    return np.float32((w * per_state).sum())


class Runner:
    """Caches the compiled PJRT executable for repeated SPMD runs."""

    def __init__(self, nc, n_cores=NCORES, n_inner=1):
        import jax
        from jax.experimental.shard_map import shard_map
        from jax.sharding import Mesh, PartitionSpec
        from concourse import bass2jax

        bass2jax.install_neuronx_cc_hook()
        self.jax = jax
        self.nc = nc
        self.n_cores = n_cores

        partition_name = (nc.partition_id_tensor.name
                          if nc.partition_id_tensor else None)
        in_names, out_names, out_avals, zero_outs = [], [], [], []
        for alloc in nc.m.functions[0].allocations:
            if not isinstance(alloc, mybir.MemoryLocationSet):
                continue
            name = alloc.memorylocations[0].name
            if alloc.kind == "ExternalInput":
                if name != partition_name:
                    in_names.append(name)
            elif alloc.kind == "ExternalOutput":
                shape = tuple(alloc.tensor_shape)
                dtype = mybir.dt.np(alloc.dtype)
                out_names.append(name)
                out_avals.append(jax.core.ShapedArray(shape, dtype))
                zero_outs.append(np.zeros(shape, dtype))
        self.in_names, self.out_names = in_names, out_names
        self.out_avals, self.zero_outs = out_avals, zero_outs
        n_params = len(in_names)
        all_in_names = list(in_names) + list(out_names)
        if partition_name is not None:
            all_in_names.append(partition_name)

        def _body(*args):
            ins = list(args[:n_params])
            cur_zeros = list(args[n_params:n_params + len(out_names)])
            extra = ([bass2jax.partition_id_tensor()]
                     if partition_name is not None else [])
            outs = tuple(cur_zeros)
            for _ in range(n_inner):
                # chain outputs into the next call's output buffers: keeps
                # every invocation live (no CSE/DCE) and is a no-op since
                # the kernel fully overwrites its outputs
                outs = bass2jax._bass_exec_p.bind(
                    *ins, *outs, *extra,
                    out_avals=tuple(out_avals),
                    in_names=tuple(all_in_names),
                    out_names=tuple(out_names),
                    lowering_input_output_aliases=(),
                    sim_require_finite=True,
                    sim_require_nnan=True,
                    nc=nc,
                )
            return tuple(outs)

        devices = jax.devices()[:n_cores]
        assert len(devices) == n_cores
        self.mesh = Mesh(np.asarray(devices), ("core",))
        self.pspec = PartitionSpec("core")
        n_outs = len(out_names)
        in_specs = (self.pspec,) * (n_params + n_outs)
        out_specs = (self.pspec,) * n_outs
        donate = tuple(range(n_params, n_params + n_outs))
        self.fn = jax.jit(
            shard_map(_body, mesh=self.mesh, in_specs=in_specs,
                      out_specs=out_specs, check_rep=False),
            donate_argnums=donate, keep_unused=True)

    def concat_inputs(self, in_maps):
        return [np.concatenate([np.asarray(in_maps[c][n])
                                for c in range(self.n_cores)], axis=0)
                for n in self.in_names]

    def device_put(self, concat_in):
        from jax.sharding import NamedSharding
        sh = NamedSharding(self.mesh, self.pspec)
        return [self.jax.device_put(a, sh) for a in concat_in]

    def run_dev(self, dev_args):
        zeros = [np.zeros((self.n_cores * z.shape[0], *z.shape[1:]), z.dtype)
                 for z in self.zero_outs]
        out = self.fn(*dev_args, *zeros)
        return self.jax.block_until_ready(out)

    def run(self, in_maps):
        out_arrs = self.run_dev(self.device_put(self.concat_inputs(in_maps)))
        return [
            {n: np.asarray(out_arrs[i]).reshape(
                self.n_cores, *self.out_avals[i].shape)[c]
             for i, n in enumerate(self.out_names)}
            for c in range(self.n_cores)
        ]


_CACHE = {}


def get_runner(variant=DEFAULT_VARIANT, reps=1, n_inner=1, **kw):
    key = (variant, reps, n_inner, tuple(sorted(kw.items())))
    if key not in _CACHE:
        if variant in ("uw16", "uw8"):
            nc = build_program_v2(reps=reps, st8=(variant == "uw8"), **kw)
        else:
            nc = build_program(variant=variant, reps=reps, **kw)
        _CACHE[key] = Runner(nc, n_inner=n_inner)
    return _CACHE[key]


def kernel(states_x, x_target, molecule_id, num_molecules):
    runner = get_runner(DEFAULT_VARIANT)
    in_maps, V = host_prep(states_x, x_target, molecule_id, num_molecules,
                           variant=DEFAULT_VARIANT)
    results = runner.run(in_maps)
    return combine(results, V)

